# revision 1
# baseline (speedup 1.0000x reference)
"""Trainium2 Bass kernel for nn_CorrelationModule (B=4, C=64, H=W=64).

Per batch b (q = query[b].reshape(C,N), ex = exemplar[b].reshape(C,N), N=4096):
  ex_corr = (W_lin @ ex)^T                      # [N, C]
  A       = ex_corr @ q                         # [N, N]
  Sm      = softmax(A, axis=-1)                 # row softmax
  att     = q @ Sm^T                            # [C, N]
  out     = leaky_relu(BN(conv3x3(att)), 0.1)

Sharding: 8 cores = (batch b, image-half h). Each core computes att for its
2048 output pixels plus one 64-pixel halo row on each side (S=2176 pixel
slice), then convolves locally. No collectives.

Per-core compute (i = pixel in slice, j = key pixel, d/c = channels), split
into 5 column phases (4x512 + 128) so the exp ops are wide (amortizing the
~352-cycle ACT fixed cost) while PSUM stays within 8 banks:
  excT[d,i]  = W_lin @ ex_slice       (fp32 matmul, duplicated to 128 parts)
  A'[j,i]    = q[:,j]^T . excT[:,i]   (fp32r; the two j-tiles of a pair are
                                       packed concurrently via 64-row
                                       tile_position groups into the two
                                       banks of one [128, 1024] PSUM tile)
  P'[j,i]    = exp(A')                (one ScalarE op per j-pair per phase,
                                       PSUM->SBUF, fp32r out; no max
                                       subtraction needed: |A| < 60 in fp32)
  att_ps     = [qT | 1]^T @ P'        (fp32r, K=128, accumulated over j in
                                       PSUM; row 64 = softmax denominator)
  att        = att_ps[0:64] * bcast(mask / att_ps[64])   (DVE + GpSimd)
  conv       = 9 accumulated fp16 matmuls over a zero-padded [65,34,66]
               buffer whose partition 64 is ones (carries the BN bias via
               weight row 64); BN scale folded into the weights host-side.
  out        = max(x, 0.1 x)          (DVE)
The 128-wide halo phase runs in bf16 (its error only touches the conv
contribution of the outermost rows).
"""

import numpy as np

B, C, H, W = 4, 64, 64, 64
N = H * W
HALF = N // 2            # 2048 output pixels per core
S = HALF + 2 * W         # 2176 slice incl. halo rows
NJT = N // 128           # 32 j-tiles
NP = NJT // 2            # 16 j-tile pairs
PH = [(0, 512), (512, 512), (1024, 512), (1536, 384), (1920, 256)]
EPS = 1e-5
N_CORES = 8

_cache = {}


def _build(n_iters=0):
    """Build+compile the SPMD module. n_iters>0 wraps the body in a HW loop
    (benchmark mode)."""
    import concourse.bacc as bacc
    import concourse.tile as tile
    from concourse import mybir
    from concourse.bass import ts

    F32 = mybir.dt.float32
    R = mybir.dt.float32r
    F16 = mybir.dt.float16
    BF = mybir.dt.bfloat16
    Exp = mybir.ActivationFunctionType.Exp
    MUL = mybir.AluOpType.mult
    MAX = mybir.AluOpType.max

    nc = bacc.Bacc("TRN2", target_bir_lowering=False, debug=False,
                   num_devices=N_CORES)
    exs_d = nc.dram_tensor("exs", [C, S], F32, kind="ExternalInput").ap()
    q_d = nc.dram_tensor("q", [C, N], R, kind="ExternalInput").ap()
    qTa_d = nc.dram_tensor("qTa", [128, NJT * 65], R, kind="ExternalInput").ap()
    W2_d = nc.dram_tensor("W2", [C, 128], F32, kind="ExternalInput").ap()
    w9_d = nc.dram_tensor("w9", [65, 9 * 64], F16, kind="ExternalInput").ap()
    mask_d = nc.dram_tensor("mask", [1, S], F32, kind="ExternalInput").ap()
    y_d = nc.dram_tensor("yout", [C, HALF], F32, kind="ExternalOutput").ap()

    def body(cp, Pp, mp, ap, sp):
        # ---- input loads (q sliced per j-pair so compute starts early) ----
        W2_t = cp.tile([C, 128], F32, name="W2_t", tag="W2_t")
        nc.sync.dma_start(out=W2_t, in_=W2_d)
        exs_t = cp.tile([C, S], F32, name="exs_t", tag="exs_t", bufs=2)
        nc.sync.dma_start(out=exs_t, in_=exs_d)
        qdup_t = cp.tile([128, N], R, name="qdup_t", tag="qdup_t", bufs=2)
        for k in range(2):
            nc.sync.dma_start(out=qdup_t[0:64, ts(k, N // 2)],
                              in_=q_d[:, ts(k, N // 2)])
            nc.vector.tensor_copy(out=qdup_t[64:128, ts(k, N // 2)],
                                  in_=qdup_t[0:64, ts(k, N // 2)])
        qTa_t = cp.tile([128, NJT, 65], R, name="qTa_t", tag="qTa_t", bufs=2)
        for k in range(2):
            nc.sync.dma_start(out=qTa_t[:, ts(k, NJT // 2), :],
                              in_=qTa_d[:, ts(k, (NJT // 2) * 65)]
                              .rearrange("p (a b) -> p a b", b=65))
        w9_t = cp.tile([65, 9, 64], F16, name="w9_t", tag="w9_t")
        nc.sync.dma_start(out=w9_t, in_=w9_d.rearrange("p (a b) -> p a b", b=64))
        mask_t = cp.tile([1, S], F32, name="mask_t", tag="mask_t")
        nc.sync.dma_start(out=mask_t, in_=mask_d)

        # ---- excT = (W_lin @ ex) duplicated onto both partition halves ----
        excT_t = cp.tile([128, S], R, name="excT_t", tag="excT_t", bufs=2)
        for c0 in range(0, S, 512):
            cw = min(512, S - c0)
            pe = mp.tile([128, cw], F32, name="pe_mm0", tag="pa")
            nc.tensor.matmul(pe, W2_t, exs_t[:, c0:c0 + cw],
                             start=True, stop=True)
            nc.vector.tensor_copy(out=excT_t[:, c0:c0 + cw], in_=pe)

        rs64 = sp.tile([65, S], F32, name="rs64", tag="rs64")
        rs0 = sp.tile([1, S], F32, name="rs0", tag="rs0")
        rb = sp.tile([64, S], F32, name="rb", tag="rb")
        pbuf = sp.tile([65, 34, W + 2], F16, name="pbuf", tag="pbuf")
        nc.vector.memset(pbuf[0:64, :, :], 0.0)
        nc.vector.memset(pbuf[64:65, :, :], 1.0)

        # ---- attention phases ----
        for pi, (p0, pw) in enumerate(PH):
            att_p = ap.tile([65, pw], F32, name=f"att{pi}", tag="att")
            for t in range(NP):
                jta, jtb = 2 * t, 2 * t + 1
                # packed pair via 64-row tile_position groups; the two halves
                # land at bank-aligned offsets 0 and 512 of one PSUM tile
                Aab = mp.tile([128, 1024], F32, name="Aab", tag="pa")
                nc.tensor.matmul(Aab[:, 0:pw],
                                 qdup_t[0:64, ts(jta, 128)],
                                 excT_t[0:64, p0:p0 + pw],
                                 start=True, stop=True)
                nc.tensor.matmul(Aab[:, 512:512 + pw],
                                 qdup_t[64:128, ts(jtb, 128)],
                                 excT_t[64:128, p0:p0 + pw],
                                 start=True, stop=True)
                Pab = Pp.tile([128, 2, pw], R, name="Pab", tag="Pab")
                nc.scalar.activation(
                    Pab,
                    Aab.rearrange("p (g x) -> p g x", g=2)[:, :, 0:pw],
                    Exp)
                nc.tensor.matmul(att_p, qTa_t[:, jta, :], Pab[:, 0, :],
                                 start=(t == 0), stop=False)
                nc.tensor.matmul(att_p, qTa_t[:, jtb, :], Pab[:, 1, :],
                                 start=False, stop=(t == NP - 1))

            # phase tail: denominator -> masked reciprocal -> broadcast ->
            # normalized write into the padded conv buffer
            sl = slice(p0, p0 + pw)
            nc.vector.reciprocal(out=rs64[64:65, sl], in_=att_p[64:65, :])
            nc.sync.dma_start(out=rs0[:, sl], in_=rs64[64:65, sl])
            nc.vector.tensor_tensor(out=rs0[:, sl], in0=rs0[:, sl],
                                    in1=mask_t[:, sl], op=MUL)
            nc.gpsimd.partition_broadcast(rb[:, sl], rs0[0:1, sl])
            r0, nr = p0 // W, pw // W
            nc.vector.tensor_tensor(
                out=pbuf[0:64, r0:r0 + nr, 1:W + 1],
                in0=att_p[0:64, :].rearrange("p (r c) -> p r c", c=W),
                in1=rb[:, sl].rearrange("p (r c) -> p r c", c=W),
                op=MUL,
            )

        # ---- conv3x3 (+BN bias via ones row) + leaky relu + store ----
        for t4 in range(4):
            r0 = 1 + 8 * t4
            pc = mp.tile([64, 512], F32, name="pc_conv", tag="pa")
            for tap in range(9):
                dy, dx = tap // 3, tap % 3
                nc.tensor.matmul(
                    pc, w9_t[:, tap, :],
                    pbuf[:, r0 - 1 + dy:r0 + 7 + dy, dx:dx + W],
                    start=(tap == 0), stop=(tap == 8),
                )
            y1 = sp.tile([64, 512], F32, name="y1", tag="y1", bufs=2)
            nc.vector.tensor_scalar_mul(y1, pc, 0.1)
            yo = sp.tile([64, 512], F32, name="yo", tag="yo", bufs=2)
            nc.vector.tensor_tensor(out=yo, in0=pc, in1=y1, op=MAX)
            nc.sync.dma_start(out=y_d[:, ts(t4, 512)], in_=yo)


    with tile.TileContext(nc) as tc:
        with tc.tile_pool(name="cp", bufs=1) as cp, \
             tc.tile_pool(name="Pp", bufs=4) as Pp, \
             tc.tile_pool(name="mp", bufs=3, space="PSUM") as mp, \
             tc.tile_pool(name="ap", bufs=2, space="PSUM") as ap, \
             tc.tile_pool(name="sp", bufs=1) as sp:
            if n_iters > 0:
                with tc.For_i(0, n_iters, 1):
                    body(cp, Pp, mp, ap, sp)
            else:
                body(cp, Pp, mp, ap, sp)

    nc.compile()
    return nc


def _prep_in_maps(exemplar, query, W_lin, conv_w, gamma, beta, run_mean,
                  run_var):
    exemplar = np.asarray(exemplar, dtype=np.float32)
    query = np.asarray(query, dtype=np.float32)
    W_lin = np.asarray(W_lin, dtype=np.float32)
    conv_w = np.asarray(conv_w, dtype=np.float32)
    gamma = np.asarray(gamma, dtype=np.float32)
    beta = np.asarray(beta, dtype=np.float32)
    run_mean = np.asarray(run_mean, dtype=np.float32)
    run_var = np.asarray(run_var, dtype=np.float32)

    s = gamma / np.sqrt(run_var + EPS)               # [C]
    t = beta - run_mean * s                          # [C]
    # w9[i, 3*dy+dx, o] = conv_w[o, i, dy, dx] * s[o]; row 64 of center tap = t
    w9 = np.zeros((65, 9, 64), dtype=np.float32)
    ws = conv_w * s[:, None, None, None]             # [o, i, 3, 3]
    w9[0:64] = ws.transpose(1, 2, 3, 0).reshape(64, 9, 64)
    w9[64, 4, :] = t
    w9 = w9.reshape(65, 9 * 64).astype(np.float16)

    W2 = np.concatenate([W_lin.T, W_lin.T], axis=1)  # [C, 128]

    in_maps = []
    for core in range(N_CORES):
        b, h = core // 2, core % 2
        ex_flat = exemplar[b].reshape(C, N)
        q_flat = query[b].reshape(C, N)
        i_lo = h * HALF - W
        s0, s1 = max(0, i_lo), min(N, i_lo + S)
        exs = np.zeros((C, S), dtype=np.float32)
        exs[:, s0 - i_lo:s1 - i_lo] = ex_flat[:, s0:s1]
        mask = np.zeros((1, S), dtype=np.float32)
        mask[0, s0 - i_lo:s1 - i_lo] = 1.0
        qTa = np.empty((128, NJT, 65), dtype=np.float32)
        qTa[:, :, 0:64] = q_flat.T.reshape(NJT, 128, 64).transpose(1, 0, 2)
        qTa[:, :, 64] = 1.0
        in_maps.append({
            "exs": exs,
            "q": np.ascontiguousarray(q_flat),
            "qTa": np.ascontiguousarray(qTa.reshape(128, NJT * 65)),
            "W2": np.ascontiguousarray(W2),
            "w9": np.ascontiguousarray(w9),
            "mask": mask,
        })
    return in_maps


def _run(in_maps, n_iters=0):
    from concourse import bass_utils
    key = ("nc", n_iters)
    if key not in _cache:
        _cache[key] = _build(n_iters)
    nc = _cache[key]
    return bass_utils.run_bass_kernel_spmd(nc, in_maps,
                                           core_ids=list(range(N_CORES)))


def kernel(exemplar, query, W_lin, conv_w, gamma, beta, run_mean, run_var):
    in_maps = _prep_in_maps(exemplar, query, W_lin, conv_w, gamma, beta,
                            run_mean, run_var)
    res = _run(in_maps)
    out = np.empty((B, C, H, W), dtype=np.float32)
    for core in range(N_CORES):
        b, h = core // 2, core % 2
        out[b, :, h * 32:(h + 1) * 32, :] = \
            res.results[core]["yout"].reshape(C, 32, W)
    return out

